# revision 8
# baseline (speedup 1.0000x reference)
"""Trainium2 Bass kernel for nn_InterClassContrastiveLoss.

Reference math (B=8192, D=512, P=N=2, T=0.1):
    f = normalize(features)                 # L2, rows
    sim = f @ f.T / T                       # [B, B]
    ps = sim[i, pos_idx[i]]; ns = sim[i, neg_idx[i]]
    loss = mean over valid pairs of softplus(ns_j - ps_p)

Key observation: only 4 entries of each sim row are used, so the B x B GEMM
is unnecessary. Per row we gather the 4 indexed feature rows and compute
    sim(i,j) = <x_i, x_j> / (|x_i| |x_j| T)
directly from raw (unnormalized) rows.

Distribution: data-parallel over the batch. Core k owns rows
[k*1024, (k+1)*1024). Every core receives the full feature table in DRAM
(input staging - no device collective needed) and gathers the rows its
indices point at via dma_gather. Per core:
  - dots on DVE via fused tensor_tensor_reduce (one pass per 128-row block
    per stream)
  - squared norms on ACT via Square + accum_out
  - rsqrt via exp(-0.5*ln(n)) (Rsqrt/Reciprocal ACT funcs are banned;
    Ln/Exp share one activation table set)
  - softplus via ln(1 + exp(10*x)) (same table set)
  - masked total + valid count reduced across partitions with a tiny
    TensorE matmul against ones
Output per core: [loss_sum, valid_count]. Host sums the 8 partials and
divides (the unshard step for a mean-reduction loss).
"""

import sys

import numpy as np

if "/opt/trn_rl_repo" not in sys.path:
    sys.path.insert(0, "/opt/trn_rl_repo")

import concourse.bacc as bacc
import concourse.bass as bass
import concourse.tile as tile
from concourse import bass_utils, mybir

B, D = 8192, 512
NCORES = 8
ROWS = B // NCORES          # rows per core
P = 128                     # SBUF partitions
NBLK = ROWS // P            # 128-row blocks per core
NSTR = 4                    # gathered streams: pos0, pos1, neg0, neg1
GSPLIT = 2                  # dma_gathers per stream (pipelining granularity)
GROWS = ROWS // GSPLIT      # indices per dma_gather
IDXC = ROWS // 16           # idx columns per stream (wrapped 16-partition layout)
INV_T = 10.0                # 1 / temperature

# knobs for test harness (ignored by graders calling kernel() directly)
TRACE = False
LAST = {}

_prog = None


def _build_program():
    nc = bacc.Bacc(
        "TRN2",
        target_bir_lowering=False,
        debug=False,
        enable_asserts=False,
        num_devices=NCORES,
    )
    FT = mybir.dt.float32
    AF = mybir.ActivationFunctionType
    OP = mybir.AluOpType

    ftab = nc.dram_tensor("ftab", [B, D], FT, kind="ExternalInput").ap()
    floc = nc.dram_tensor("floc", [ROWS, D], FT, kind="ExternalInput").ap()
    gidx = nc.dram_tensor(
        "gidx", [P, NSTR * IDXC], mybir.dt.int16, kind="ExternalInput"
    ).ap()
    vmf = nc.dram_tensor("vmf", [P, NBLK], FT, kind="ExternalInput").ap()
    outt = nc.dram_tensor("out", [2, 1], FT, kind="ExternalOutput").ap()

    def bc(ap, n, axis=1):
        """Insert a step-0 (broadcast) dim of size n after the partition dim."""
        aps = [list(x) for x in ap.ap]
        aps.insert(axis, [0, n])
        return bass.AP(tensor=ap.tensor, offset=ap.offset, ap=aps)

    with tile.TileContext(nc) as tc:
        with tc.tile_pool(name="big", bufs=1) as big, \
             tc.tile_pool(name="small", bufs=1) as small, \
             tc.tile_pool(name="scr_a", bufs=4) as scr_a, \
             tc.tile_pool(name="scr_v", bufs=4) as scr_v, \
             tc.tile_pool(name="pp", bufs=1, space="PSUM") as pp:

            idx_sb = small.tile([P, NSTR * IDXC], mybir.dt.int16)
            nc.sync.dma_start(out=idx_sb[:], in_=gidx)
            vm_sb = small.tile([P, NBLK], FT)
            nc.sync.dma_start(out=vm_sb[:], in_=vmf)
            f_all = big.tile([P, NBLK, D], FT)
            nc.sync.dma_start(out=f_all[:], in_=floc.rearrange("(b p) d -> p b d", p=P))

            # gather the indexed rows: stream s, half h covers local rows
            # [h*GROWS, (h+1)*GROWS); dest partition = i%128, block = i//128
            gt = []
            bpg = NBLK // GSPLIT
            for s in range(NSTR):
                g = big.tile([P, NBLK, D], FT, tag=f"g{s}")
                for h in range(GSPLIT):
                    c0 = s * IDXC + h * (GROWS // 16)
                    nc.gpsimd.dma_gather(
                        out_ap=g[:, h * bpg:(h + 1) * bpg, :],
                        in_ap=ftab,
                        idxs_ap=idx_sb[:, c0:c0 + GROWS // 16],
                        num_idxs=GROWS,
                        num_idxs_reg=GROWS,
                        elem_size=D,
                    )
                gt.append(g)

            # squared norms (ACT, Square + accum_out) and raw dots (DVE,
            # elementwise mult + 3D reduce), per half-stream for pipelining
            nrm = small.tile([P, (1 + NSTR) * NBLK], FT)
            dots = small.tile([P, NSTR * NBLK], FT)
            for b in range(NBLK):
                t = scr_a.tile([P, D], FT, tag="sa")
                nc.scalar.activation(
                    out=t[:], in_=f_all[:, b, :], func=AF.Square,
                    accum_out=nrm[:, b:b + 1],
                )
            for s in range(NSTR):
                for b in range(NBLK):
                    c = (1 + s) * NBLK + b
                    t = scr_a.tile([P, D], FT, tag="sa")
                    nc.scalar.activation(
                        out=t[:], in_=gt[s][:, b, :], func=AF.Square,
                        accum_out=nrm[:, c:c + 1],
                    )
                for h in range(GSPLIT):
                    hh = scr_v.tile([P, bpg, D], FT, tag="h")
                    nc.vector.tensor_tensor(
                        out=hh[:], in0=f_all[:, h * bpg:(h + 1) * bpg, :],
                        in1=gt[s][:, h * bpg:(h + 1) * bpg, :], op=OP.mult,
                    )
                    nc.vector.tensor_reduce(
                        out=dots[:, s * NBLK + h * bpg:s * NBLK + (h + 1) * bpg],
                        in_=hh[:], axis=mybir.AxisListType.X, op=OP.add,
                    )

            # r = 1/sqrt(nrm) = exp(-0.5 * ln(max(nrm, tiny)))
            nc.vector.tensor_scalar_max(out=nrm[:], in0=nrm[:], scalar1=1e-20)
            lnrm = small.tile([P, (1 + NSTR) * NBLK], FT)
            nc.scalar.activation(out=lnrm[:], in_=nrm[:], func=AF.Ln)
            rall = small.tile([P, (1 + NSTR) * NBLK], FT)
            nc.scalar.activation(out=rall[:], in_=lnrm[:], func=AF.Exp, scale=-0.5)

            # sims[s, b] = dots[s, b] * r_self[b] * r_s[b]   (un-T-scaled)
            rpair = small.tile([P, NSTR * NBLK], FT)
            nc.vector.tensor_tensor(
                out=rpair[:].rearrange("p (s b) -> p s b", s=NSTR),
                in0=bc(rall[:, 0:NBLK], NSTR),
                in1=rall[:, NBLK:(1 + NSTR) * NBLK].rearrange(
                    "p (s b) -> p s b", s=NSTR),
                op=OP.mult,
            )
            sims = small.tile([P, NSTR * NBLK], FT)
            nc.vector.tensor_tensor(out=sims[:], in0=dots[:], in1=rpair[:], op=OP.mult)

            # diffs[j, i, b] = ns_j[b] - ps_i[b]  (4 combos)
            diffs = small.tile([P, 4 * NBLK], FT)
            ns = sims[:, 2 * NBLK:4 * NBLK]
            ps_ = sims[:, 0:2 * NBLK]
            ns_b = bass.AP(tensor=ns.tensor, offset=ns.offset,
                           ap=[list(ns.ap[0]), [NBLK, 2], [0, 2], [1, NBLK]])
            ps_b = bass.AP(tensor=ps_.tensor, offset=ps_.offset,
                           ap=[list(ps_.ap[0]), [0, 2], [NBLK, 2], [1, NBLK]])
            nc.vector.tensor_tensor(
                out=diffs[:].rearrange("p (j i b) -> p j i b", j=2, i=2),
                in0=ns_b, in1=ps_b, op=OP.subtract,
            )

            # softplus(10*diff) = ln(1 + exp(10*diff)); fp32-safe for |10*diff|<=20
            et = small.tile([P, 4 * NBLK], FT)
            nc.scalar.activation(out=et[:], in_=diffs[:], func=AF.Exp, scale=INV_T)
            u = small.tile([P, 4 * NBLK], FT)
            nc.scalar.activation(out=u[:], in_=et[:], func=AF.Ln, bias=1.0)

            # stats[:,0] = sum_j,i,b u * vm[b];  stats[:,1] = sum_b vm[b]
            stats = small.tile([P, 2], FT)
            mu = small.tile([P, 4 * NBLK], FT)
            nc.vector.tensor_tensor(
                out=mu[:].rearrange("p (c b) -> p c b", c=4),
                in0=u[:].rearrange("p (c b) -> p c b", c=4),
                in1=bc(vm_sb[:], 4), op=OP.mult,
            )
            nc.vector.tensor_reduce(
                out=stats[:, 0:1], in_=mu[:],
                axis=mybir.AxisListType.X, op=OP.add,
            )
            nc.vector.tensor_reduce(
                out=stats[:, 1:2], in_=vm_sb[:],
                axis=mybir.AxisListType.X, op=OP.add,
            )

            # partition-sum both stats columns: [2,1] = stats.T @ ones
            ones = small.tile([P, 1], FT)
            nc.vector.memset(ones[:], 1.0)
            acc = pp.tile([2, 1], FT)
            nc.tensor.matmul(out=acc[:], lhsT=stats[:], rhs=ones[:],
                             start=True, stop=True)
            res = small.tile([2, 1], FT)
            nc.vector.tensor_copy(out=res[:], in_=acc[:])
            nc.sync.dma_start(out=outt, in_=res[:])

    nc.compile()
    return nc


def _make_in_maps(feats, pidx, nidx, vld):
    idx_all = np.concatenate([pidx, nidx], axis=1)  # [B, 4]
    in_maps = []
    for k in range(NCORES):
        sl = slice(k * ROWS, (k + 1) * ROWS)
        gidx = np.zeros((P, NSTR * IDXC), np.int16)
        for s in range(NSTR):
            col = idx_all[sl, s].astype(np.int16)
            for h in range(GSPLIT):
                seg = col[h * GROWS:(h + 1) * GROWS]
                c0 = s * IDXC + h * (GROWS // 16)
                # gather position i reads its index from [i%16, c0 + i//16];
                # the 16-partition wrapped block must be replicated to all 8
                # Q7-core partition groups (HW reads per-core windows)
                gidx[:, c0:c0 + GROWS // 16] = np.tile(
                    seg.reshape(GROWS // 16, 16).T, (P // 16, 1))
        vmf = vld[sl].astype(np.float32).reshape(NBLK, P).T
        in_maps.append({
            "ftab": feats,
            "floc": np.ascontiguousarray(feats[sl]),
            "gidx": gidx,
            "vmf": np.ascontiguousarray(vmf),
        })
    return in_maps


def kernel(features, pos_idx, neg_idx, valid):
    global _prog
    feats = np.ascontiguousarray(np.asarray(features, dtype=np.float32))
    pidx = np.asarray(pos_idx).astype(np.int64)
    nidx = np.asarray(neg_idx).astype(np.int64)
    vld = np.asarray(valid).astype(bool)
    assert feats.shape == (B, D)

    if _prog is None:
        _prog = _build_program()
    nc = _prog

    in_maps = _make_in_maps(feats, pidx, nidx, vld)
    res = bass_utils.run_bass_kernel_spmd(
        nc, in_maps, core_ids=list(range(NCORES)), trace=TRACE,
    )
    LAST["res"] = res
    LAST["exec_time_ns"] = res.exec_time_ns

    tot = 0.0
    cnt = 0.0
    for k in range(NCORES):
        o = np.asarray(res.results[k]["out"], dtype=np.float64).reshape(2)
        tot += o[0]
        cnt += o[1]
    num_pairs = cnt * 4.0
    val = tot / num_pairs if num_pairs > 0 else 0.0
    return np.float32(val)
